# revision 59
# baseline (speedup 1.0000x reference)
"""Trainium2 Bass kernel for the attention-gate block (v2: n-major layout).

Math (per sample n, after folding BN into the convs):
  X     = x[n, :, ::2, ::2].reshape(C, 4)                 # C=512, L=4
  act_k = relu(Wk' @ X + bk')            k=0,1,2          # D=64 each
  S     = act0^T act1  (4x4);  P = softmax_rows(S)
  Z     = P @ act2^T  (4x64)
  Y     = W4' @ Z^T + b4'                                  # (512, 4)
  out[n,c,h,w] = x[n,c,h,w] + Y[c,h]                       # broadcast over w

Device mapping (per core, 256 samples, blocks of 128):
  - x loads/stores in n-major layout [128 samples, (c h w)=8192]: each
    partition line is one 32KB-contiguous HBM run, so DMA runs at full
    HBM rate (vs 256B-descriptor runs in a channel-packed layout).
  - the ::2,::2 subsample is brought to [c, (n l)] layout with 16 PE
    transposes per block (one per (c-group, l)); evacuated to SBUF as
    bf16 by the scalar engine.
  - all large matmuls run in bf16 (1 cycle/row vs 4 for fp32, and
    fast-weight-load applies); accumulation stays fp32 in PSUM.
  - q and k share one stationary [c,128] so GEMM1 is 4 matmuls total.
  - attention per 32-sample subchunk: block-diag gram scores, masked
    exp (ACT + mask multiply), softmax denominators via a ones-column
    matmul, reciprocal_approx_fast, normalization on the z columns.
  - GEMM2 is operand-swapped: out_l[n, c] = [z_l; denom-ones] ^T @
    [W4'; b4'], so output partitions are samples and the residual adds
    run directly against the n-major x tile (w-broadcast via a
    stride-0 AP on the PSUM operand). Residual split: DVE does l=0,1
    straight from PSUM; scalar engine evacuates l=2,3 for GPSIMD.
"""

import os
import sys

for _p in ("/opt/trn_rl_repo",):
    if _p not in sys.path:
        sys.path.insert(0, _p)

import numpy as np

import concourse.mybir as mybir
from concourse import bacc, tile

EPS = 1e-5
N_TOTAL, C, D, HH, WW = 2048, 512, 64, 4, 4
NCORES = 8
NSH = N_TOTAL // NCORES  # 256 samples per core
BLK = int(os.environ.get("KBLK", "128"))  # samples per block
SUB = 32                 # samples per attention subchunk (4*SUB = 128 cols)
SHIFT = -34.0            # constant exp shift; cancels in the normalization
F32 = mybir.dt.float32
BF16 = mybir.dt.bfloat16

_PROG_CACHE = {}

# the 4 strided-conv taps (h', w') in l order, matching reference's
# reshape(L) = h'*2 + w'
POS = [(0, 0), (0, 2), (2, 0), (2, 2)]


def build_program(nsh=NSH, blk=BLK, reps=1):
    key = (nsh, blk, reps)
    if key in _PROG_CACHE:
        return _PROG_CACHE[key]

    nc = bacc.Bacc("TRN2", target_bir_lowering=False, debug=False)
    AF = mybir.ActivationFunctionType

    x_in = nc.dram_tensor("x", (nsh, C, HH, WW), F32, kind="ExternalInput")
    wqk = nc.dram_tensor("wqk", (128, 4, 128), BF16, kind="ExternalInput")
    bqk = nc.dram_tensor("bqk", (128, 1), F32, kind="ExternalInput")
    w2a = nc.dram_tensor("w2a", (128, 4, D), BF16, kind="ExternalInput")
    b2a = nc.dram_tensor("b2a", (1, D), BF16, kind="ExternalInput")
    w4b = nc.dram_tensor("w4b", (D + 1, C), BF16, kind="ExternalInput")
    msk = nc.dram_tensor("msk", (128, 128), BF16, kind="ExternalInput")
    idn = nc.dram_tensor("idn", (128, 128), BF16, kind="ExternalInput")
    out = nc.dram_tensor("out", (nsh, C, HH, WW), F32, kind="ExternalOutput")

    nblk = nsh // blk
    nsub = (4 * blk) // 128
    NF = 4 * blk  # free width of a full block of (n, l) columns

    with tile.TileContext(nc) as tc:
        with (
            tc.tile_pool(name="const", bufs=1) as cpool,
            tc.tile_pool(name="xp", bufs=3) as xpool,
            tc.tile_pool(name="xs", bufs=2) as xspool,
            tc.tile_pool(name="work", bufs=4) as wpool,
            tc.tile_pool(name="att", bufs=6) as apool,
            tc.tile_pool(name="z", bufs=2) as zpool,
            tc.tile_pool(name="ps", bufs=4, space="PSUM") as pspool,
            tc.tile_pool(name="psy", bufs=2, space="PSUM") as pypool,
        ):
            # idn rides the sync queue ahead of the x loads (it gates
            # the first transposes); the rest of the weights ride the
            # scalar HWDGE queue, idle until the stores
            idn_sb = cpool.tile([128, 128], BF16)
            nc.sync.dma_start(idn_sb[:], idn[:])
            wqk_sb = cpool.tile([128, 4, 128], BF16)
            nc.scalar.dma_start(wqk_sb[:], wqk[:])
            w2a_sb = cpool.tile([128, 4, D], BF16)
            nc.scalar.dma_start(w2a_sb[:], w2a[:])
            b2a_sb = cpool.tile([1, D], BF16)
            nc.scalar.dma_start(b2a_sb[:], b2a[:])
            msk_sb = cpool.tile([128, 128], BF16)
            nc.scalar.dma_start(msk_sb[:], msk[:])
            bq_sb = cpool.tile([D, 1], F32)
            nc.scalar.dma_start(bq_sb[:], bqk[0:D])
            bk_sb = cpool.tile([D, 1], F32)
            nc.scalar.dma_start(bk_sb[:], bqk[D:128])
            w4b_sb = cpool.tile([D + 1, C], BF16)
            nc.scalar.dma_start(w4b_sb[:], w4b[:])
            ones_row = cpool.tile([1, 128], BF16)
            nc.vector.memset(ones_row[:], 1.0)
            ones_col = cpool.tile([128, 1], BF16)
            nc.vector.memset(ones_col[:], 1.0)
            shift_sb = cpool.tile([128, 1], F32)
            nc.vector.memset(shift_sb[:], SHIFT)

            xv = x_in[:].rearrange("(b n) c h w -> b n (c h w)", n=blk)
            ov = out[:].rearrange("(b n) c h w -> b n (c h w)", n=blk)

            blist = [b for _ in range(reps) for b in range(nblk)]

            # pre-issue every block's loads as 4 c-quarter tiles so the
            # first transposes start as soon as quarter 0 lands; casts to
            # bf16 ride along on DVE (idle during the load window) so no
            # later block's transposes queue behind residual work
            xq, xbfs = {}, {}
            for i, b in enumerate(blist):
                xbf = wpool.tile([blk, 4, 4, 128], BF16, tag="xbf",
                                 bufs=2)
                for g in range(4):
                    t = xpool.tile([blk, 2048], F32, tag=f"x{g}")
                    nc.sync.dma_start(
                        t[:], xv[b][:, g * 2048:(g + 1) * 2048])
                    xq[(i, g)] = t
                    xgv = t[:].rearrange("p (c h w) -> p c h w", h=4, w=4)
                    src = xgv[:, :, 0:4:2, 0:4:2].transpose([0, 2, 3, 1])
                    nc.vector.tensor_copy(
                        xbf[:, g].rearrange("p (a e) c -> p a e c", a=2), src)
                xbfs[i] = xbf

            for i, b in enumerate(blist):
                # PE-transpose the bf16 taps to [c, (n l)]; bf16 gets
                # fast-weight-load, 4 taps share one PSUM bank so each
                # c-group evacuates with a single strided copy
                xbf = xbfs[i]
                xsT = xspool.tile([128, 4, blk, 4], BF16, tag="xs")
                for g in range(4):
                    ps_t = pspool.tile([128, 4 * blk], BF16, tag="ps")
                    for l in range(4):
                        nc.tensor.transpose(
                            ps_t[:, l * blk:(l + 1) * blk], xbf[:, g, l],
                            idn_sb[0:blk, 0:blk])
                    ot = xsT[:, g].transpose([0, 2, 1])
                    it = ps_t[:].rearrange("p (l n) -> p l n", l=4)
                    if g % 2 == 0:
                        nc.scalar.activation(ot, it, AF.Copy)
                    else:
                        nc.vector.tensor_copy(ot, it)

                # GEMM1 q and k over 4 c-group contraction steps
                ps_q = pspool.tile([D, NF], F32, tag="ps")
                ps_k = pspool.tile([D, NF], F32, tag="ps")
                for g in range(4):
                    nc.tensor.matmul(
                        ps_q[:], lhsT=wqk_sb[:, g, 0:D],
                        rhs=xsT[:, g].rearrange("p n l -> p (n l)"),
                        start=(g == 0), stop=(g == 3),
                    )
                for g in range(4):
                    nc.tensor.matmul(
                        ps_k[:], lhsT=wqk_sb[:, g, D:128],
                        rhs=xsT[:, g].rearrange("p n l -> p (n l)"),
                        start=(g == 0), stop=(g == 3),
                    )
                a_q = wpool.tile([D, NF], BF16, tag="aq")
                nc.scalar.activation(a_q[:], ps_q[:], AF.Relu, bias=bq_sb[:])
                a_k = wpool.tile([D, NF], BF16, tag="ak")
                nc.scalar.activation(a_k[:], ps_k[:], AF.Relu, bias=bk_sb[:])

                # attention: v^T per subchunk; gram/exp/mask batched across
                # subchunks; denominators land directly in [n, l] partition
                # layout via tiny per-l column-sum matmuls, so softmax
                # division happens inside the residual STT
                z_t = zpool.tile([D + 1, NF], BF16, tag="z")
                ps_d4 = pspool.tile([blk, 4], F32, tag="d4", bufs=2)
                ps_g4 = pspool.tile([128, NF], F32, tag="ps")
                ps_vt4 = pspool.tile([128, nsub, D], F32, tag="ps")
                for s in range(nsub):
                    ns = slice(s * SUB, (s + 1) * SUB)
                    cl = slice(s * 128, s * 128 + 128)
                    for g in range(4):
                        nc.tensor.matmul(
                            ps_vt4[:, s],
                            lhsT=xsT[:, g, ns].rearrange("p n l -> p (n l)"),
                            rhs=w2a_sb[:, g],
                            start=(g == 0), stop=False,
                        )
                    nc.tensor.matmul(
                        ps_vt4[:, s], lhsT=ones_row[:], rhs=b2a_sb[:],
                        start=False, stop=True,
                    )
                    nc.tensor.matmul(
                        ps_g4[:, cl], lhsT=a_k[:, cl], rhs=a_q[:, cl],
                        start=True, stop=True,
                    )
                a2t4 = apool.tile([128, nsub, D + 1], BF16, tag="a2t")
                nc.vector.memset(a2t4[:, :, D], 1.0)
                nc.scalar.activation(a2t4[:, :, 0:D], ps_vt4[:], AF.Relu)
                e4 = apool.tile([128, NF], BF16, tag="e")
                nc.scalar.activation(e4[:], ps_g4[:], AF.Exp,
                                     bias=shift_sb[:])
                p0 = apool.tile([128, nsub, 128], BF16, tag="p0")
                mb = msk_sb[:].unsqueeze(1).broadcast_to((128, nsub, 128))
                nc.vector.tensor_mul(
                    p0[:], e4[:].rearrange("p (s c) -> p s c", s=nsub), mb)
                for s in range(nsub):
                    cl = slice(s * 128, s * 128 + 128)
                    ps_z = pspool.tile([D + 1, 128], F32, tag="ps")
                    nc.tensor.matmul(
                        ps_z[:], lhsT=a2t4[:, s], rhs=p0[:, s],
                        start=True, stop=True,
                    )
                    nc.scalar.activation(z_t[:, cl], ps_z[:], AF.Copy)
                # denominator column-sums over pairs of subchunks (matmul
                # out base partition must be 0/32/64; the (s, n) slice
                # merges to one free dim since n-stride*32 == s-stride)
                p0v = p0[:].rearrange("p s (n l) -> p s n l", l=4)
                for h in range((nsub + 1) // 2):
                    sl = slice(2 * h, min(2 * h + 2, nsub))
                    nh = (sl.stop - sl.start) * SUB
                    for l in range(4):
                        nc.tensor.matmul(
                            ps_d4[h * 2 * SUB:h * 2 * SUB + nh, l:l + 1],
                            lhsT=p0v[:, sl, :, l], rhs=ones_col[:],
                            start=True, stop=True,
                        )
                rinv4 = apool.tile([blk, 4], F32, tag="rinv")
                nc.vector.reciprocal_approx_fast(rinv4[:], ps_d4[:])

                # GEMM2 (operand-swapped; denom row of z scales the bias
                # correctly since softmax division is deferred to the STT)
                zv = z_t[:].rearrange("p (n l) -> p n l", l=4)
                ps_ys, y_sbs = {}, {}
                for l in (3, 2, 0, 1):
                    ps_y = pypool.tile([blk, C], F32, tag="psy")
                    nc.tensor.matmul(
                        ps_y[:], lhsT=zv[:, :, l], rhs=w4b_sb[:],
                        start=True, stop=True,
                    )
                    ps_ys[l] = ps_y
                    if l >= 2:
                        # evacuate pre-scaled by 1/denom so GPSIMD can do a
                        # plain add (TensorScalarPtr is not a Pool-engine op)
                        y_sb = wpool.tile([blk, C], F32, tag="ysb")
                        nc.scalar.activation(y_sb[:], ps_y[:], AF.Copy,
                                             scale=rinv4[:, l:l + 1])
                        y_sbs[l] = y_sb

                # residual out = x + y*rinv, quarter-major so each store
                # leaves early; DVE covers l=0,1 from PSUM (plus l=2 on
                # the first two quarters), GPSIMD the evacuated rest
                for g in range(4):
                    xgv = xq[(i, g)][:].rearrange(
                        "p (c h w) -> p c h w", h=4, w=4)
                    cg = slice(g * 128, (g + 1) * 128)
                    for l in range(4):
                        if l < 2:
                            src = (ps_ys[l][:, cg]
                                   .unsqueeze(2).broadcast_to((blk, 128, 4)))
                            nc.vector.scalar_tensor_tensor(
                                xgv[:, :, l, :], src, rinv4[:, l:l + 1],
                                xgv[:, :, l, :],
                                op0=mybir.AluOpType.mult,
                                op1=mybir.AluOpType.add,
                            )
                        else:
                            src = (y_sbs[l][:, cg]
                                   .unsqueeze(2).broadcast_to((blk, 128, 4)))
                            eng = nc.vector if (l == 2 and g < 2) else \
                                nc.gpsimd
                            eng.tensor_add(
                                xgv[:, :, l, :], xgv[:, :, l, :], src)
                    nc.sync.dma_start(
                        ov[b][:, g * 2048:(g + 1) * 2048], xq[(i, g)][:])

    nc.compile()
    _PROG_CACHE[key] = nc
    return nc


def prep_params(W123, b123, g123, be123, m123, v123, W4, b4, g4, be4, m4, v4):
    """Fold BN into the convs; arrange weights for the device layout."""
    import ml_dtypes

    f32 = np.float32
    bf16 = ml_dtypes.bfloat16
    s123 = (g123 / np.sqrt(v123 + EPS)).astype(f32)            # (3, D)
    Wf = (W123 * s123[:, :, None]).astype(f32)                 # (3, D, C)
    bf = ((b123 - m123) * s123 + be123).astype(f32)            # (3, D)
    s4 = (g4 / np.sqrt(v4 + EPS)).astype(f32)                  # (C,)
    W4f = (W4 * s4[:, None]).astype(f32)                       # (C, D)
    b4f = ((b4 - m4) * s4 + be4).astype(f32)                   # (C,)

    # [c, d] -> [g=4, p=128, d] -> [p, g, d] with c = g*128 + p
    def by_group(wt):  # wt: (C, d)
        return np.ascontiguousarray(
            wt.reshape(4, 128, -1).transpose(1, 0, 2))

    wqk = by_group(np.concatenate([Wf[0].T, Wf[1].T], axis=1))  # (128,4,128)
    bqk = np.concatenate([bf[0], bf[1]])[:, None]               # (128, 1)
    w2a = by_group(Wf[2].T)                                     # (128, 4, 64)
    b2a = bf[2][None, :]                                        # (1, D)
    w4b = np.concatenate([W4f.T, b4f[None, :]], axis=0)         # (65, C)
    msk = np.kron(np.eye(SUB, dtype=f32), np.ones((4, 4), f32))  # (128, 128)
    idn = np.eye(128, dtype=f32)
    return dict(
        wqk=wqk.astype(bf16), bqk=np.ascontiguousarray(bqk, f32),
        w2a=w2a.astype(bf16), b2a=np.ascontiguousarray(b2a).astype(bf16),
        w4b=np.ascontiguousarray(w4b).astype(bf16), msk=msk.astype(bf16),
        idn=idn.astype(bf16),
    )


def _run(inputs, trace=False, **spmd_kwargs):
    from concourse.bass_utils import run_bass_kernel_spmd

    x = np.ascontiguousarray(np.asarray(inputs["x"], dtype=np.float32))
    params = prep_params(**{k: np.asarray(v, np.float64)
                            for k, v in inputs.items() if k != "x"})
    nc = build_program()
    in_maps = [
        {"x": x[i * NSH:(i + 1) * NSH], **params} for i in range(NCORES)
    ]
    res = run_bass_kernel_spmd(
        nc, in_maps, list(range(NCORES)), trace=trace, **spmd_kwargs
    )
    outs = np.concatenate(
        [np.asarray(res.results[i]["out"]) for i in range(NCORES)], axis=0
    )
    return outs, res


def kernel(**inputs):
    outs, _ = _run(inputs)
    return outs


# revision 61
# speedup vs baseline: 1.1884x; 1.1884x over previous
"""Trainium2 Bass kernel for the attention-gate block (v2: n-major layout).

Math (per sample n, after folding BN into the convs):
  X     = x[n, :, ::2, ::2].reshape(C, 4)                 # C=512, L=4
  act_k = relu(Wk' @ X + bk')            k=0,1,2          # D=64 each
  S     = act0^T act1  (4x4);  P = softmax_rows(S)
  Z     = P @ act2^T  (4x64)
  Y     = W4' @ Z^T + b4'                                  # (512, 4)
  out[n,c,h,w] = x[n,c,h,w] + Y[c,h]                       # broadcast over w

Device mapping (per core, 256 samples, blocks of 128):
  - x loads/stores in n-major layout [128 samples, (c h w)=8192]: each
    partition line is one 32KB-contiguous HBM run, so DMA runs at full
    HBM rate (vs 256B-descriptor runs in a channel-packed layout).
  - the ::2,::2 subsample is brought to [c, (n l)] layout with 16 PE
    transposes per block (one per (c-group, l)); evacuated to SBUF as
    bf16 by the scalar engine.
  - all large matmuls run in bf16 (1 cycle/row vs 4 for fp32, and
    fast-weight-load applies); accumulation stays fp32 in PSUM.
  - q and k share one stationary [c,128] so GEMM1 is 4 matmuls total.
  - attention per 32-sample subchunk: block-diag gram scores, masked
    exp (ACT + mask multiply), softmax denominators via a ones-column
    matmul, reciprocal_approx_fast, normalization on the z columns.
  - GEMM2 is operand-swapped: out_l[n, c] = [z_l; denom-ones] ^T @
    [W4'; b4'], so output partitions are samples and the residual adds
    run directly against the n-major x tile (w-broadcast via a
    stride-0 AP on the PSUM operand). Residual split: DVE does l=0,1
    straight from PSUM; scalar engine evacuates l=2,3 for GPSIMD.
"""

import os
import sys

for _p in ("/opt/trn_rl_repo",):
    if _p not in sys.path:
        sys.path.insert(0, _p)

import numpy as np

import concourse.mybir as mybir
from concourse import bacc, tile

EPS = 1e-5
N_TOTAL, C, D, HH, WW = 2048, 512, 64, 4, 4
NCORES = 8
NSH = N_TOTAL // NCORES  # 256 samples per core
BLK = int(os.environ.get("KBLK", "128"))  # samples per block
SUB = 32                 # samples per attention subchunk (4*SUB = 128 cols)
SHIFT = -34.0            # constant exp shift; cancels in the normalization
F32 = mybir.dt.float32
BF16 = mybir.dt.bfloat16

_PROG_CACHE = {}

# the 4 strided-conv taps (h', w') in l order, matching reference's
# reshape(L) = h'*2 + w'
POS = [(0, 0), (0, 2), (2, 0), (2, 2)]


def build_program(nsh=NSH, blk=BLK, reps=1):
    key = (nsh, blk, reps)
    if key in _PROG_CACHE:
        return _PROG_CACHE[key]

    nc = bacc.Bacc("TRN2", target_bir_lowering=False, debug=False)
    AF = mybir.ActivationFunctionType

    x_in = nc.dram_tensor("x", (nsh, C, HH, WW), F32, kind="ExternalInput")
    wqk = nc.dram_tensor("wqk", (128, 4, 128), BF16, kind="ExternalInput")
    bqk = nc.dram_tensor("bqk", (128, 1), F32, kind="ExternalInput")
    w2a = nc.dram_tensor("w2a", (128, 4, D), BF16, kind="ExternalInput")
    b2a = nc.dram_tensor("b2a", (1, D), BF16, kind="ExternalInput")
    w4b = nc.dram_tensor("w4b", (D + 1, C), BF16, kind="ExternalInput")
    msk = nc.dram_tensor("msk", (128, 128), BF16, kind="ExternalInput")
    idn = nc.dram_tensor("idn", (128, 128), BF16, kind="ExternalInput")
    out = nc.dram_tensor("out", (nsh, C, HH, WW), F32, kind="ExternalOutput")

    nblk = nsh // blk
    nsub = (4 * blk) // 128
    NF = 4 * blk  # free width of a full block of (n, l) columns

    with tile.TileContext(nc) as tc:
        with (
            tc.tile_pool(name="const", bufs=1) as cpool,
            tc.tile_pool(name="xp", bufs=3) as xpool,
            tc.tile_pool(name="xs", bufs=2) as xspool,
            tc.tile_pool(name="work", bufs=4) as wpool,
            tc.tile_pool(name="att", bufs=6) as apool,
            tc.tile_pool(name="z", bufs=2) as zpool,
            tc.tile_pool(name="ps", bufs=3, space="PSUM") as pspool,
            tc.tile_pool(name="psy", bufs=3, space="PSUM") as pypool,
        ):
            # idn rides the sync queue ahead of the x loads (it gates
            # the first transposes); the rest of the weights ride the
            # scalar HWDGE queue, idle until the stores
            idn_sb = cpool.tile([128, 128], BF16)
            nc.sync.dma_start(idn_sb[:], idn[:])
            wqk_sb = cpool.tile([128, 4, 128], BF16)
            nc.scalar.dma_start(wqk_sb[:], wqk[:])
            w2a_sb = cpool.tile([128, 4, D], BF16)
            nc.scalar.dma_start(w2a_sb[:], w2a[:])
            b2a_sb = cpool.tile([1, D], BF16)
            nc.scalar.dma_start(b2a_sb[:], b2a[:])
            msk_sb = cpool.tile([128, 128], BF16)
            nc.scalar.dma_start(msk_sb[:], msk[:])
            bq_sb = cpool.tile([D, 1], F32)
            nc.scalar.dma_start(bq_sb[:], bqk[0:D])
            bk_sb = cpool.tile([D, 1], F32)
            nc.scalar.dma_start(bk_sb[:], bqk[D:128])
            w4b_sb = cpool.tile([D + 1, C], BF16)
            nc.scalar.dma_start(w4b_sb[:], w4b[:])
            ones_row = cpool.tile([1, 128], BF16)
            nc.vector.memset(ones_row[:], 1.0)
            ones_col = cpool.tile([128, 1], BF16)
            nc.vector.memset(ones_col[:], 1.0)
            shift_sb = cpool.tile([128, 1], F32)
            nc.vector.memset(shift_sb[:], SHIFT)

            xv = x_in[:].rearrange("(b n) c h w -> b n (c h w)", n=blk)
            ov = out[:].rearrange("(b n) c h w -> b n (c h w)", n=blk)

            blist = [b for _ in range(reps) for b in range(nblk)]

            # pre-issue every block's loads as 4 c-quarter tiles so the
            # first transposes start as soon as quarter 0 lands; casts to
            # bf16 ride along on DVE (idle during the load window) so no
            # later block's transposes queue behind residual work
            xq, xbfs = {}, {}
            for i, b in enumerate(blist):
                xbf = wpool.tile([blk, 4, 4, 128], BF16, tag="xbf",
                                 bufs=2)
                for g in range(4):
                    t = xpool.tile([blk, 2048], F32, tag=f"x{g}")
                    nc.sync.dma_start(
                        t[:], xv[b][:, g * 2048:(g + 1) * 2048])
                    xq[(i, g)] = t
                    xgv = t[:].rearrange("p (c h w) -> p c h w", h=4, w=4)
                    src = xgv[:, :, 0:4:2, 0:4:2].transpose([0, 2, 3, 1])
                    nc.vector.tensor_copy(
                        xbf[:, g].rearrange("p (a e) c -> p a e c", a=2), src)
                xbfs[i] = xbf

            for i, b in enumerate(blist):
                # PE-transpose the bf16 taps to [c, (n l)]; bf16 gets
                # fast-weight-load, 4 taps share one PSUM bank so each
                # c-group evacuates with a single strided copy
                xbf = xbfs[i]
                xsT = xspool.tile([128, 4, blk, 4], BF16, tag="xs")
                for g in range(4):
                    ps_t = pspool.tile([128, 4 * blk], BF16, tag="ps")
                    for l in range(4):
                        nc.tensor.transpose(
                            ps_t[:, l * blk:(l + 1) * blk], xbf[:, g, l],
                            idn_sb[0:blk, 0:blk])
                    ot = xsT[:, g].transpose([0, 2, 1])
                    it = ps_t[:].rearrange("p (l n) -> p l n", l=4)
                    if g % 2 == 0:
                        nc.scalar.activation(ot, it, AF.Copy)
                    else:
                        nc.vector.tensor_copy(ot, it)

                # GEMM1 q and k over 4 c-group contraction steps
                ps_q = pspool.tile([D, NF], F32, tag="ps")
                ps_k = pspool.tile([D, NF], F32, tag="ps")
                for g in range(4):
                    nc.tensor.matmul(
                        ps_q[:], lhsT=wqk_sb[:, g, 0:D],
                        rhs=xsT[:, g].rearrange("p n l -> p (n l)"),
                        start=(g == 0), stop=(g == 3),
                    )
                for g in range(4):
                    nc.tensor.matmul(
                        ps_k[:], lhsT=wqk_sb[:, g, D:128],
                        rhs=xsT[:, g].rearrange("p n l -> p (n l)"),
                        start=(g == 0), stop=(g == 3),
                    )
                a_q = wpool.tile([D, NF], BF16, tag="aq")
                nc.scalar.activation(a_q[:], ps_q[:], AF.Relu, bias=bq_sb[:])
                a_k = wpool.tile([D, NF], BF16, tag="ak")
                nc.scalar.activation(a_k[:], ps_k[:], AF.Relu, bias=bk_sb[:])

                # attention: v^T per subchunk; gram/exp/mask batched across
                # subchunks; denominators land directly in [n, l] partition
                # layout via tiny per-l column-sum matmuls, so softmax
                # division happens inside the residual STT
                z_t = zpool.tile([D + 1, NF], BF16, tag="z")
                ps_d4 = pspool.tile([blk, 4], F32, tag="d4", bufs=2)
                ps_g4 = pspool.tile([128, NF], F32, tag="ps")
                ps_vt4 = pspool.tile([128, nsub, D], F32, tag="ps")
                for s in range(nsub):
                    ns = slice(s * SUB, (s + 1) * SUB)
                    cl = slice(s * 128, s * 128 + 128)
                    for g in range(4):
                        nc.tensor.matmul(
                            ps_vt4[:, s],
                            lhsT=xsT[:, g, ns].rearrange("p n l -> p (n l)"),
                            rhs=w2a_sb[:, g],
                            start=(g == 0), stop=False,
                        )
                    nc.tensor.matmul(
                        ps_vt4[:, s], lhsT=ones_row[:], rhs=b2a_sb[:],
                        start=False, stop=True,
                    )
                    nc.tensor.matmul(
                        ps_g4[:, cl], lhsT=a_k[:, cl], rhs=a_q[:, cl],
                        start=True, stop=True,
                    )
                a2t4 = apool.tile([128, nsub, D + 1], BF16, tag="a2t")
                nc.vector.memset(a2t4[:, :, D], 1.0)
                nc.scalar.activation(a2t4[:, :, 0:D], ps_vt4[:], AF.Relu)
                e4 = apool.tile([128, NF], BF16, tag="e")
                nc.scalar.activation(e4[:], ps_g4[:], AF.Exp,
                                     bias=shift_sb[:])
                p0 = apool.tile([128, nsub, 128], BF16, tag="p0")
                mb = msk_sb[:].unsqueeze(1).broadcast_to((128, nsub, 128))
                nc.vector.tensor_mul(
                    p0[:], e4[:].rearrange("p (s c) -> p s c", s=nsub), mb)
                for s in range(nsub):
                    cl = slice(s * 128, s * 128 + 128)
                    ps_z = pspool.tile([D + 1, 128], F32, tag="ps")
                    nc.tensor.matmul(
                        ps_z[:], lhsT=a2t4[:, s], rhs=p0[:, s],
                        start=True, stop=True,
                    )
                    nc.scalar.activation(z_t[:, cl], ps_z[:], AF.Copy)
                # denominator column-sums over pairs of subchunks (matmul
                # out base partition must be 0/32/64; the (s, n) slice
                # merges to one free dim since n-stride*32 == s-stride)
                p0v = p0[:].rearrange("p s (n l) -> p s n l", l=4)
                for h in range((nsub + 1) // 2):
                    sl = slice(2 * h, min(2 * h + 2, nsub))
                    nh = (sl.stop - sl.start) * SUB
                    for l in range(4):
                        nc.tensor.matmul(
                            ps_d4[h * 2 * SUB:h * 2 * SUB + nh, l:l + 1],
                            lhsT=p0v[:, sl, :, l], rhs=ones_col[:],
                            start=True, stop=True,
                        )
                rinv4 = apool.tile([blk, 4], F32, tag="rinv")
                nc.vector.reciprocal_approx_fast(rinv4[:], ps_d4[:])

                # GEMM2 (operand-swapped; denom row of z scales the bias
                # correctly since softmax division is deferred to the STT)
                zv = z_t[:].rearrange("p (n l) -> p n l", l=4)
                ps_ys, y_sbs = {}, {}
                for l in (3, 2, 0, 1):
                    ps_y = pypool.tile([blk, C], F32, tag="psy")
                    nc.tensor.matmul(
                        ps_y[:], lhsT=zv[:, :, l], rhs=w4b_sb[:],
                        start=True, stop=True,
                    )
                    ps_ys[l] = ps_y
                    if l >= 2:
                        # evacuate pre-scaled by 1/denom so GPSIMD can do a
                        # plain add (TensorScalarPtr is not a Pool-engine op)
                        y_sb = wpool.tile([blk, C], F32, tag="ysb")
                        nc.scalar.activation(y_sb[:], ps_y[:], AF.Copy,
                                             scale=rinv4[:, l:l + 1])
                        y_sbs[l] = y_sb

                # residual out = x + y*rinv, quarter-major so each store
                # leaves early; DVE covers l=0,1 from PSUM (plus l=2 on
                # the first two quarters), GPSIMD the evacuated rest
                for g in range(4):
                    xgv = xq[(i, g)][:].rearrange(
                        "p (c h w) -> p c h w", h=4, w=4)
                    cg = slice(g * 128, (g + 1) * 128)
                    for l in range(4):
                        if l < 2 or (l == 2 and g < 2):
                            src = (ps_ys[l][:, cg]
                                   .unsqueeze(2).broadcast_to((blk, 128, 4)))
                            nc.vector.scalar_tensor_tensor(
                                xgv[:, :, l, :], src, rinv4[:, l:l + 1],
                                xgv[:, :, l, :],
                                op0=mybir.AluOpType.mult,
                                op1=mybir.AluOpType.add,
                            )
                        else:
                            src = (y_sbs[l][:, cg]
                                   .unsqueeze(2).broadcast_to((blk, 128, 4)))
                            nc.gpsimd.tensor_add(
                                xgv[:, :, l, :], xgv[:, :, l, :], src)
                    nc.sync.dma_start(
                        ov[b][:, g * 2048:(g + 1) * 2048], xq[(i, g)][:])

    nc.compile()
    _PROG_CACHE[key] = nc
    return nc


def prep_params(W123, b123, g123, be123, m123, v123, W4, b4, g4, be4, m4, v4):
    """Fold BN into the convs; arrange weights for the device layout."""
    import ml_dtypes

    f32 = np.float32
    bf16 = ml_dtypes.bfloat16
    s123 = (g123 / np.sqrt(v123 + EPS)).astype(f32)            # (3, D)
    Wf = (W123 * s123[:, :, None]).astype(f32)                 # (3, D, C)
    bf = ((b123 - m123) * s123 + be123).astype(f32)            # (3, D)
    s4 = (g4 / np.sqrt(v4 + EPS)).astype(f32)                  # (C,)
    W4f = (W4 * s4[:, None]).astype(f32)                       # (C, D)
    b4f = ((b4 - m4) * s4 + be4).astype(f32)                   # (C,)

    # [c, d] -> [g=4, p=128, d] -> [p, g, d] with c = g*128 + p
    def by_group(wt):  # wt: (C, d)
        return np.ascontiguousarray(
            wt.reshape(4, 128, -1).transpose(1, 0, 2))

    wqk = by_group(np.concatenate([Wf[0].T, Wf[1].T], axis=1))  # (128,4,128)
    bqk = np.concatenate([bf[0], bf[1]])[:, None]               # (128, 1)
    w2a = by_group(Wf[2].T)                                     # (128, 4, 64)
    b2a = bf[2][None, :]                                        # (1, D)
    w4b = np.concatenate([W4f.T, b4f[None, :]], axis=0)         # (65, C)
    msk = np.kron(np.eye(SUB, dtype=f32), np.ones((4, 4), f32))  # (128, 128)
    idn = np.eye(128, dtype=f32)
    return dict(
        wqk=wqk.astype(bf16), bqk=np.ascontiguousarray(bqk, f32),
        w2a=w2a.astype(bf16), b2a=np.ascontiguousarray(b2a).astype(bf16),
        w4b=np.ascontiguousarray(w4b).astype(bf16), msk=msk.astype(bf16),
        idn=idn.astype(bf16),
    )


def _run(inputs, trace=False, **spmd_kwargs):
    from concourse.bass_utils import run_bass_kernel_spmd

    x = np.ascontiguousarray(np.asarray(inputs["x"], dtype=np.float32))
    params = prep_params(**{k: np.asarray(v, np.float64)
                            for k, v in inputs.items() if k != "x"})
    nc = build_program()
    in_maps = [
        {"x": x[i * NSH:(i + 1) * NSH], **params} for i in range(NCORES)
    ]
    res = run_bass_kernel_spmd(
        nc, in_maps, list(range(NCORES)), trace=trace, **spmd_kwargs
    )
    outs = np.concatenate(
        [np.asarray(res.results[i]["out"]) for i in range(NCORES)], axis=0
    )
    return outs, res


def kernel(**inputs):
    outs, _ = _run(inputs)
    return outs
